# revision 6
# baseline (speedup 1.0000x reference)
"""Trainium2 Bass kernel for the CT-metrics pairwise MLP scorer (fp8 DoubleRow).

Computes, for M_aug [N,D] and Q [M,D] (N=M=512, D=1024):
    diff2[n,m,:] = (M_aug[n]-Q[m])**2
    cost[n,m]    = diff2.sum(-1)
    d[n,m]       = -(MLP(diff2[n,m,:]) + b3)      (D->512->256->1, leaky relu)
    bw           = softmax(d, axis=0)
    score[m]     = sum_n cost*bw,  score_fg[m] = sum_{n<N_fg} cost*bw

Sharding: N axis split across 8 cores (64 rows each).  Per-core partial
column sums S1 = sum_n exp(d), S1fg, Sc = sum_n exp(d)*(||Mn||^2 - 2 Mn.Q)
and Scfg; host combine adds ||Q||^2 and normalizes (flash-softmax style,
logits are O(1) so no max shift needed).

Speed strategy vs the bf16 baseline (577us): all MLP matmuls are fp8e4m3
DoubleRow (2 contraction chunks per instruction at 0.5 cyc/row = 4x bf16
FLOP rate), so the PE drops to ~3.4us/row-of-64 equivalent and the bound
moves to the elementwise engines.  Layer-1 moving operands are built per
row with ONE elementwise op per 128-chunk via
    W1 . diff2  =  (2*W1) . X + [W1 . Mn^2 folded into bias]
    X = qt*mt + 0.5*qt^2   (DVE fused scalar_tensor_tensor, 6 chunks)
with the remaining 2 chunks in square form split ACT/GPSIMD (GPSIMD
cannot run scalar_tensor_tensor or read PSUM).  h1/h2 lrelu PSUM drains
are fused pair-instructions on ACT with NO bias operand: the b1+A and b2
biases are accumulated into PSUM by rank-1 fp8 matmuls (stationary
[1,128] bias slice on partition 0 x moving ones row), which makes the
drains fusable.  Engine busy per row (ns): DVE ~3564, ACT ~3420 avg,
GPSIMD ~2880 avg, PE ~3414.
"""

from contextlib import ExitStack

import numpy as np

import concourse.bass as bass
import concourse.tile as tile
from concourse import bacc, mybir
from concourse.bass_utils import run_bass_kernel_spmd

N_CORES = 8
N, M, D, H = 512, 512, 1024, 512
K2 = H // 2  # 256
NL = N // N_CORES  # 64 rows per core
DC, HC, KC, MC = D // 128, H // 128, K2 // 128, M // 128  # 8, 4, 2, 4
NEG_SLOPE = 0.01

F32 = mybir.dt.float32
BF16 = mybir.dt.bfloat16
F8 = mybir.dt.float8e4
AF = mybir.ActivationFunctionType
DR = mybir.MatmulPerfMode.DoubleRow
ALU = mybir.AluOpType

# Per-chunk X producer, alternating by row parity to level engine load:
# 'v' = DVE fused X-form, 'p' = GPSIMD 2-op square, 'a' = ACT square.
X_SPLIT_EVEN = "vvvvvvap"
X_SPLIT_ODD = "vvvvvvpp"
# chunks that EVER use the X-form (must be static: sets W1 scale + A bias)
_V_CHUNKS = [c == "v" for c in X_SPLIT_EVEN]

# f32 packed tensor column offsets
_MT0 = 0                      # mt [128, DC, NL] f32 (= -M rows, scalars)
_C00 = _MT0 + DC * NL         # -b3 column [128, 1]
_MN0 = _C00 + 1               # row0: ||Mn||^2 [1, NL]
_FG0 = _MN0 + NL              # row0: fg mask tiled [1, MC*NL]
_ON0 = _FG0 + MC * NL         # row0: ones [1, 128]
_PA_COLS = _ON0 + 128

# bf16 packed: qt | qh | mtc2
_QT0 = 0
_QH0 = _QT0 + DC * M          # 4096
_MC0 = _QH0 + DC * M          # 8192
_PQ_COLS = _MC0 + DC * NL     # 8704

# fp8 packed (all partitions): w1 | w2 | w3
_W10 = 0
_W20 = _W10 + DC * H          # 4096
_W30 = _W20 + HC * K2         # 5120
_P8_COLS = _W30 + KC          # 5122

# fp8 packed partition-0 row: A' slices | b2 slices | ones
_A80 = 0                      # (n*HC + hc)*128
_B80 = _A80 + NL * HC * 128   # 32768
_O80 = _B80 + KC * 128        # 33024
_PZ_COLS = _O80 + 512         # 33536


def emit_body(nc, tc, ctx, pa_sb, pq_sb, p8_sb, pz_sb, stats):
    def pool(name, bufs, space="SBUF"):
        return ctx.enter_context(tc.tile_pool(name=name, bufs=bufs, space=space))

    diffp = pool("diffp", 2)
    tvp = pool("tvp", 2)
    h1p = pool("h1p", 2)
    h2p = pool("h2p", 2)
    ep = pool("ep", 1)
    l1ps = pool("l1ps", 2, "PSUM")   # [128,2,512] pair tiles, 4 banks
    l2ps = pool("l2ps", 1, "PSUM")   # [128,2,512], 2 banks
    l3ps = pool("l3ps", 1, "PSUM")   # 1 bank
    gps = pool("gps", 1, "PSUM")     # 1 bank (g_t, then mask)
    consts = pool("km_consts", 1)

    mt_sb = pa_sb[:, _MT0:_C00].rearrange("p (c n) -> p c n", c=DC)
    c0_sb = pa_sb[:, _C00:_C00 + 1]
    mn2h = pa_sb[0:1, _MN0:_MN0 + NL]
    fg_row = pa_sb[0:1, _FG0:_FG0 + MC * NL]
    onesr = pa_sb[0:1, _ON0:_ON0 + 128]
    qt_sb = pq_sb[:, _QT0:_QH0].rearrange("p (c m) -> p c m", c=DC)
    qh_sb = pq_sb[:, _QH0:_MC0].rearrange("p (c m) -> p c m", c=DC)
    mtc2 = pq_sb[:, _MC0:_PQ_COLS].rearrange("p (c n) -> p c n", c=DC)
    w1_sb = p8_sb[:, _W10:_W20].rearrange("p (c h) -> p c h", c=DC)
    w2_sb = p8_sb[:, _W20:_W30].rearrange("p (c k) -> p c k", c=HC)
    w3p = p8_sb[:, _W30:_P8_COLS].rearrange("p (c o) -> p c o", c=KC)
    ones8 = pz_sb[0:1, _O80:_O80 + 512]

    def a8_slice(n, hc):
        off = _A80 + (n * HC + hc) * 128
        return pz_sb[0:1, off:off + 128]

    def b8_slice(kc):
        off = _B80 + kc * 128
        return pz_sb[0:1, off:off + 128]

    # ---- prologue: cost[n,m] partial = ||Mn||^2 - 2 Mn.Q  (transposed to
    # [m-part, mc, n]) and fg mask broadcast ----
    g_t = gps.tile([128, MC, NL], F32, tag="g_t")
    for mc in range(MC):
        for dc in range(DC):
            nc.tensor.matmul(g_t[:, mc, :],
                             qt_sb[:, dc, mc * 128:(mc + 1) * 128],
                             mtc2[:, dc, :],
                             start=(dc == 0), stop=False)
        nc.tensor.matmul(g_t[:, mc, :], onesr, mn2h, start=False, stop=True)
    cost_t = consts.tile([128, MC, NL], F32, tag="cost_t")
    nc.vector.tensor_copy(cost_t[:], g_t[:])

    mask_ps = gps.tile([128, MC, NL], F32, tag="g_t")
    nc.tensor.matmul(mask_ps[:].rearrange("p c n -> p (c n)"), onesr, fg_row,
                     start=True, stop=True)
    mask_bc = consts.tile([128, MC, NL], F32, tag="mask_bc")
    nc.vector.tensor_copy(mask_bc[:], mask_ps[:])

    d_ps = l3ps.tile([128, MC, NL], F32, tag="d_ps")

    # ---- main loop over local rows ----
    for n in range(NL):
        x_split = X_SPLIT_EVEN if n % 2 == 0 else X_SPLIT_ODD
        x8 = diffp.tile([128, DC, M], F8, tag="x8")
        for dc in range(DC):
            eng = x_split[dc]
            if eng == "v":
                nc.vector.scalar_tensor_tensor(
                    x8[:, dc, :], qt_sb[:, dc, :], mt_sb[:, dc, n:n + 1],
                    qh_sb[:, dc, :], ALU.mult, ALU.add)
            elif eng == "p":
                tv = tvp.tile([128, M], BF16, tag="tv")
                nc.gpsimd.tensor_scalar_add(tv[:], qt_sb[:, dc, :],
                                            mt_sb[:, dc, n:n + 1])
                nc.gpsimd.tensor_mul(x8[:, dc, :], tv[:], tv[:])
            else:
                nc.scalar.activation(x8[:, dc, :], qt_sb[:, dc, :], AF.Square,
                                     bias=mt_sb[:, dc, n:n + 1])

        h1 = h1p.tile([128, HC, M], F8, tag="h1")
        for hp in range(HC // 2):
            p1 = l1ps.tile([128, 2, M], F32, tag="p1")
            for j in range(2):
                hc = 2 * hp + j
                for dp in range(DC // 2):
                    nc.tensor.matmul(
                        p1[:, j, :],
                        w1_sb[:, 2 * dp:2 * dp + 2, hc * 128:(hc + 1) * 128],
                        x8[:, 2 * dp:2 * dp + 2, :],
                        start=(dp == 0), stop=False, perf_mode=DR)
                nc.tensor.matmul(p1[:, j, :], a8_slice(n, hc), ones8,
                                 start=False, stop=True)
            nc.scalar.activation(h1[:, 2 * hp:2 * hp + 2, :], p1[:],
                                 AF.Lrelu, alpha=NEG_SLOPE)

        p2 = l2ps.tile([128, KC, M], F32, tag="p2")
        for kc in range(KC):
            for hp in range(HC // 2):
                nc.tensor.matmul(
                    p2[:, kc, :],
                    w2_sb[:, 2 * hp:2 * hp + 2, kc * 128:(kc + 1) * 128],
                    h1[:, 2 * hp:2 * hp + 2, :],
                    start=(hp == 0), stop=False, perf_mode=DR)
            nc.tensor.matmul(p2[:, kc, :], b8_slice(kc), ones8,
                             start=False, stop=True)
        h2 = h2p.tile([128, KC, M], F8, tag="h2")
        nc.scalar.activation(h2[:], p2[:], AF.Lrelu, alpha=NEG_SLOPE)

        for mc in range(MC):
            nc.tensor.matmul(
                d_ps[:, mc, n:n + 1],
                h2[:, 0:2, mc * 128:(mc + 1) * 128],
                w3p[:, 0:2, :],
                start=True, stop=True, perf_mode=DR)

    # ---- epilogue ----
    e_t = ep.tile([128, MC, NL], F32, tag="e_t")
    nc.scalar.activation(e_t[:], d_ps[:], AF.Exp, bias=c0_sb, scale=-1.0)
    w_t = ep.tile([128, MC, NL], F32, tag="w_t")
    nc.vector.tensor_mul(w_t[:], e_t[:], cost_t[:])
    efg_t = ep.tile([128, MC, NL], F32, tag="efg_t")
    nc.gpsimd.tensor_mul(efg_t[:], e_t[:], mask_bc[:])
    wfg_t = ep.tile([128, MC, NL], F32, tag="wfg_t")
    nc.gpsimd.tensor_mul(wfg_t[:], w_t[:], mask_bc[:])

    stats_sb = consts.tile([128, 4, MC], F32, tag="stats_sb")
    for s, src in ((0, e_t), (1, efg_t), (2, w_t), (3, wfg_t)):
        nc.vector.tensor_reduce(stats_sb[:, s, :], src[:],
                                axis=mybir.AxisListType.X,
                                op=mybir.AluOpType.add)
    nc.sync.dma_start(stats[:], stats_sb[:])


def build_program():
    nc = bacc.Bacc("TRN2", target_bir_lowering=False, debug=False,
                   num_devices=N_CORES)
    pa = nc.dram_tensor("pa", [128, _PA_COLS], F32, kind="ExternalInput").ap()
    pq = nc.dram_tensor("pq", [128, _PQ_COLS], BF16, kind="ExternalInput").ap()
    p8 = nc.dram_tensor("p8", [128, _P8_COLS], F8, kind="ExternalInput").ap()
    pz = nc.dram_tensor("pz", [1, _PZ_COLS], F8, kind="ExternalInput").ap()
    stats = nc.dram_tensor("stats", [128, 4, MC], F32,
                           kind="ExternalOutput").ap()

    with tile.TileContext(nc) as tc, ExitStack() as ctx:
        consts = ctx.enter_context(tc.tile_pool(name="consts", bufs=1))
        pa_sb = consts.tile([128, _PA_COLS], F32, tag="pa_sb")
        nc.sync.dma_start(pa_sb[:], pa[:])
        pq_sb = consts.tile([128, _PQ_COLS], BF16, tag="pq_sb")
        nc.sync.dma_start(pq_sb[:, 0:_QH0], pq[:, 0:_QH0])
        p8_sb = consts.tile([128, _P8_COLS], F8, tag="p8_sb")
        nc.sync.dma_start(p8_sb[:], p8[:])
        pz_sb = consts.tile([128, _PZ_COLS], F8, tag="pz_sb")
        nc.sync.dma_start(pz_sb[0:1, :], pz[:])
        nc.sync.dma_start(pq_sb[:, _QH0:_PQ_COLS], pq[:, _QH0:_PQ_COLS])
        emit_body(nc, tc, ctx, pa_sb, pq_sb, p8_sb, pz_sb, stats)

    nc.compile()
    return nc


def shard_inputs(M_aug, Q, W1, b1, W2, b2, W3, b3, N_fg):
    """Host-side layout prep. Returns per-core input maps."""
    import ml_dtypes
    f = np.float32
    bf = ml_dtypes.bfloat16
    e4 = ml_dtypes.float8_e4m3
    M_aug = np.asarray(M_aug, np.float64)
    Q = np.asarray(Q, np.float64)
    W1 = np.asarray(W1, np.float64)
    W2 = np.asarray(W2, np.float64)
    W3 = np.asarray(W3, np.float64)
    b1 = np.asarray(b1, np.float64)
    b2 = np.asarray(b2, np.float64)
    b3 = np.asarray(b3, np.float64)
    nfg = int(N_fg)

    def part_major(a2d, chunks):  # [C*128, F] -> [128, C*F]
        cdim, fdim = a2d.shape
        assert cdim == chunks * 128
        return np.ascontiguousarray(
            a2d.reshape(chunks, 128, fdim).transpose(1, 0, 2)).reshape(128, -1)

    v_chunks = np.array(_V_CHUNKS)
    w1_scale = np.where(np.repeat(v_chunks, 128), 2.0, 1.0)  # [D]
    v_dmask = np.repeat(v_chunks, 128).astype(np.float64)     # [D]

    p8_v = np.zeros((128, _P8_COLS), e4)
    p8_v[:, _W10:_W20] = part_major((W1 * w1_scale[None, :]).T, DC).astype(e4)
    p8_v[:, _W20:_W30] = part_major(W2.T, HC).astype(e4)
    p8_v[:, _W30:_P8_COLS] = W3.reshape(KC, 128).T.astype(e4)
    p8_v = np.ascontiguousarray(p8_v)

    qt = part_major(Q.T, DC).astype(bf)                  # [128, DC*M] bf16
    qh = (0.5 * qt.astype(np.float64) ** 2).astype(bf)

    base = np.zeros((128, _PA_COLS), f)
    base[:, _C00] = -float(b3[0])
    base[0, _ON0:_ON0 + 128] = 1.0

    in_maps = []
    for c in range(N_CORES):
        rows = slice(c * NL, (c + 1) * NL)
        Mrows = M_aug[rows]                               # [NL, D]
        pa_v = base.copy()
        pa_v[:, _MT0:_C00] = part_major(-Mrows.T, DC)
        pa_v[0, _MN0:_MN0 + NL] = (Mrows ** 2).sum(-1)
        gidx = np.arange(c * NL, (c + 1) * NL)
        pa_v[0, _FG0:_FG0 + MC * NL] = np.tile((gidx < nfg).astype(f), MC)

        # A' bias: b1 + sum_{d in v-chunks} W1[h,d]*M[n,d]^2, [H, NL]
        Ap = (b1[:, None] + W1 @ (v_dmask[:, None] * (Mrows.T ** 2)))
        pz_v = np.zeros((1, _PZ_COLS), e4)
        # (n*HC + hc)*128 layout == Ap.T [NL, H] flattened
        pz_v[0, _A80:_B80] = np.ascontiguousarray(Ap.T).reshape(-1).astype(e4)
        pz_v[0, _B80:_O80] = b2.astype(e4)
        pz_v[0, _O80:_O80 + 512] = 1.0

        pq_v = np.zeros((128, _PQ_COLS), bf)
        pq_v[:, _QT0:_QH0] = qt
        pq_v[:, _QH0:_MC0] = qh
        pq_v[:, _MC0:_PQ_COLS] = part_major(-2.0 * Mrows.T, DC).astype(bf)
        in_maps.append({"pa": pa_v, "pq": np.ascontiguousarray(pq_v),
                        "p8": p8_v, "pz": pz_v})
    return in_maps


def combine(stats_list, Q):
    """stats_list: per-core [128, 4, MC] arrays -> (score, score_fg)."""
    st = np.stack([
        np.asarray(s, np.float64).transpose(1, 2, 0).reshape(4, M)
        for s in stats_list
    ])  # [C, 4, M]
    S1 = st[:, 0].sum(0)
    S1fg = st[:, 1].sum(0)
    Sc = st[:, 2].sum(0)
    Scfg = st[:, 3].sum(0)
    qn2 = (np.asarray(Q, np.float64) ** 2).sum(-1)
    score = Sc / S1 + qn2
    score_fg = Scfg / S1 + qn2 * (S1fg / S1)
    return score.astype(np.float32), score_fg.astype(np.float32)


_PROGRAM_CACHE = {}


def run(trace=False, **inputs):
    if "prog" not in _PROGRAM_CACHE:
        _PROGRAM_CACHE["prog"] = build_program()
    nc = _PROGRAM_CACHE["prog"]
    in_maps = shard_inputs(**inputs)
    res = run_bass_kernel_spmd(nc, in_maps, list(range(N_CORES)), trace=trace)
    outs = combine([res.results[c]["stats"] for c in range(N_CORES)],
                   inputs["Q"])
    return outs, res


def kernel(**inputs):
    outs, _ = run(trace=False, **inputs)
    return outs


# revision 15
# speedup vs baseline: 1.0442x; 1.0442x over previous
"""Trainium2 Bass kernel for the CT-metrics pairwise MLP scorer (fp8 DoubleRow).

Computes, for M_aug [N,D] and Q [M,D] (N=M=512, D=1024):
    diff2[n,m,:] = (M_aug[n]-Q[m])**2
    cost[n,m]    = diff2.sum(-1)
    d[n,m]       = -(MLP(diff2[n,m,:]) + b3)      (D->512->256->1, leaky relu)
    bw           = softmax(d, axis=0)
    score[m]     = sum_n cost*bw,  score_fg[m] = sum_{n<N_fg} cost*bw

Sharding: N axis split across 8 cores (64 rows each).  Per-core partial
column sums S1 = sum_n exp(d), S1fg, Sc = sum_n exp(d)*(||Mn||^2 - 2 Mn.Q)
and Scfg; host combine adds ||Q||^2 and normalizes (flash-softmax style,
logits are O(1) so no max shift needed).

Speed strategy vs the bf16 baseline (577us): all MLP matmuls are fp8e4m3
DoubleRow (2 contraction chunks per instruction at 0.5 cyc/row = 4x bf16
FLOP rate), so the PE drops to ~3.4us/row-of-64 equivalent and the bound
moves to the elementwise engines.  Layer-1 moving operands are built per
row with ONE elementwise op per 128-chunk via
    W1 . diff2  =  (2*W1) . X + [W1 . Mn^2 folded into bias]
    X = qt*mt + 0.5*qt^2   (DVE fused scalar_tensor_tensor, 6 chunks)
with the remaining 2 chunks in square form split ACT/GPSIMD (GPSIMD
cannot run scalar_tensor_tensor or read PSUM).  h1/h2 lrelu PSUM drains
are fused pair-instructions on ACT with NO bias operand: the b1+A and b2
biases are accumulated into PSUM by rank-1 fp8 matmuls (stationary
[1,128] bias slice on partition 0 x moving ones row), which makes the
drains fusable.  Engine busy per row (ns): DVE ~3564, ACT ~3420 avg,
GPSIMD ~2880 avg, PE ~3414.
"""

from contextlib import ExitStack

import numpy as np

import concourse.bass as bass
import concourse.tile as tile
from concourse import bacc, mybir
from concourse.bass_utils import run_bass_kernel_spmd

N_CORES = 8
N, M, D, H = 512, 512, 1024, 512
K2 = H // 2  # 256
NL = N // N_CORES  # 64 rows per core
DC, HC, KC, MC = D // 128, H // 128, K2 // 128, M // 128  # 8, 4, 2, 4
NEG_SLOPE = 0.01

F32 = mybir.dt.float32
BF16 = mybir.dt.bfloat16
F8 = mybir.dt.float8e4
AF = mybir.ActivationFunctionType
DR = mybir.MatmulPerfMode.DoubleRow
ALU = mybir.AluOpType

# Per-chunk X producer, alternating by row parity to level engine load:
# 'v' = DVE fused X-form, 'p' = GPSIMD 2-op square, 'a' = ACT square.
X_SPLIT_EVEN = "vvvvvvap"
X_SPLIT_ODD = "vvvvvvpp"
# chunks that EVER use the X-form (must be static: sets W1 scale + A bias)
_V_CHUNKS = [c == "v" for c in X_SPLIT_EVEN]

# f32 packed tensor column offsets
_MT0 = 0                      # mt [128, DC, NL] f32 (= -M rows, scalars)
_C00 = _MT0 + DC * NL         # -b3 column [128, 1]
_MN0 = _C00 + 1               # row0: ||Mn||^2 [1, NL]
_FG0 = _MN0 + NL              # row0: fg mask tiled [1, MC*NL]
_ON0 = _FG0 + MC * NL         # row0: ones [1, 128]
_PA_COLS = _ON0 + 128

# bf16 packed: qt | qh | mtc2
_QT0 = 0
_QH0 = _QT0 + DC * M          # 4096
_MC0 = _QH0 + DC * M          # 8192
_PQ_COLS = _MC0 + DC * NL     # 8704

# fp8 packed (all partitions): w1 | w2 | w3
_W10 = 0
_W20 = _W10 + DC * H          # 4096
_W30 = _W20 + HC * K2         # 5120
_P8_COLS = _W30 + KC          # 5122

# fp8 packed partition-0 row: A' pair slices | b2 pair slices | ones pair
# (pairs are [data(128) | zeros(128)] so the rank-1 bias matmuls can run in
# DoubleRow mode at 0.5 cyc/row)
_A80 = 0                      # (n*HC + hc)*256
_B80 = _A80 + NL * HC * 256   # 65536
_O80 = _B80 + KC * 256        # 66048
_PZ_COLS = _O80 + 1024        # 67072


def emit_body(nc, tc, ctx, pa_sb, pq_sb, p8_sb, pz_sb, stats):
    def pool(name, bufs, space="SBUF"):
        return ctx.enter_context(tc.tile_pool(name=name, bufs=bufs, space=space))

    xp = pool("xp", 8)
    tvp = pool("tvp", 3)
    h1p = pool("h1p", 2)
    h2p = pool("h2p", 2)
    ep = pool("ep", 1)
    l1ps = pool("l1ps", 2, "PSUM")   # [128,2,512] pair tiles, 4 banks
    l2ps = pool("l2ps", 1, "PSUM")   # [128,2,512], 2 banks
    l3ps = pool("l3ps", 1, "PSUM")   # 1 bank
    gps = pool("gps", 1, "PSUM")     # 1 bank (g_t, then mask)
    consts = pool("km_consts", 1)

    mt_sb = pa_sb[:, _MT0:_C00].rearrange("p (c n) -> p c n", c=DC)
    c0_sb = pa_sb[:, _C00:_C00 + 1]
    mn2h = pa_sb[0:1, _MN0:_MN0 + NL]
    fg_row = pa_sb[0:1, _FG0:_FG0 + MC * NL]
    onesr = pa_sb[0:1, _ON0:_ON0 + 128]
    qt_sb = pq_sb[:, _QT0:_QH0].rearrange("p (c m) -> p c m", c=DC)
    qh_sb = pq_sb[:, _QH0:_MC0].rearrange("p (c m) -> p c m", c=DC)
    mtc2 = pq_sb[:, _MC0:_PQ_COLS].rearrange("p (c n) -> p c n", c=DC)
    w1_sb = p8_sb[:, _W10:_W20].rearrange("p (c h) -> p c h", c=DC)
    w2_sb = p8_sb[:, _W20:_W30].rearrange("p (c k) -> p c k", c=HC)
    w3p = p8_sb[:, _W30:_P8_COLS].rearrange("p (c o) -> p c o", c=KC)
    ones8p = pz_sb[0:1, _O80:_O80 + 1024].rearrange("p (a b) -> p a b", a=2)

    def a8_slice(n, hc):
        off = _A80 + (n * HC + hc) * 256
        return pz_sb[0:1, off:off + 256].rearrange("p (a b) -> p a b", a=2)

    def b8_slice(kc):
        off = _B80 + kc * 256
        return pz_sb[0:1, off:off + 256].rearrange("p (a b) -> p a b", a=2)

    # ---- prologue: cost[n,m] partial = ||Mn||^2 - 2 Mn.Q  (transposed to
    # [m-part, mc, n]) and fg mask broadcast ----
    g_t = gps.tile([128, MC, NL], F32, tag="g_t")
    for mc in range(MC):
        for dc in range(DC):
            nc.tensor.matmul(g_t[:, mc, :],
                             qt_sb[:, dc, mc * 128:(mc + 1) * 128],
                             mtc2[:, dc, :],
                             start=(dc == 0), stop=False)
        nc.tensor.matmul(g_t[:, mc, :], onesr, mn2h, start=False, stop=True)
    cost_t = consts.tile([128, MC, NL], F32, tag="cost_t")
    nc.vector.tensor_copy(cost_t[:], g_t[:])

    mask_ps = gps.tile([128, MC, NL], F32, tag="g_t")
    nc.tensor.matmul(mask_ps[:].rearrange("p c n -> p (c n)"), onesr, fg_row,
                     start=True, stop=True)
    mask_bc = consts.tile([128, MC, NL], F32, tag="mask_bc")
    nc.vector.tensor_copy(mask_bc[:], mask_ps[:])

    d_ps = l3ps.tile([128, MC, NL], F32, tag="d_ps")

    # ---- main loop over local rows ----
    for n in range(NL):
        x_split = X_SPLIT_EVEN if n % 2 == 0 else X_SPLIT_ODD
        xpair = []
        for dp in range(DC // 2):
            xpair.append(xp.tile([128, 2, M], F8, tag="x8",
                                 name=f"x8_{n}_{dp}"))
        for dc in range(DC):
            eng = x_split[dc]
            xdst = xpair[dc // 2][:, dc % 2, :]
            if eng == "v":
                nc.vector.scalar_tensor_tensor(
                    xdst, qt_sb[:, dc, :], mt_sb[:, dc, n:n + 1],
                    qh_sb[:, dc, :], ALU.mult, ALU.add)
            elif eng == "p":
                tv = tvp.tile([128, M], BF16, tag="tv")
                nc.gpsimd.tensor_scalar_add(tv[:], qt_sb[:, dc, :],
                                            mt_sb[:, dc, n:n + 1])
                nc.gpsimd.tensor_mul(xdst, tv[:], tv[:])
            else:
                nc.scalar.activation(xdst, qt_sb[:, dc, :], AF.Square,
                                     bias=mt_sb[:, dc, n:n + 1])

        h1 = h1p.tile([128, HC, M], F8, tag="h1")
        for hp in range(HC // 2):
            p1 = l1ps.tile([128, 2, M], F32, tag="p1")
            for j in range(2):
                hc = 2 * hp + j
                for dp in range(DC // 2):
                    nc.tensor.matmul(
                        p1[:, j, :],
                        w1_sb[:, 2 * dp:2 * dp + 2, hc * 128:(hc + 1) * 128],
                        xpair[dp][:],
                        start=(dp == 0), stop=False, perf_mode=DR)
                nc.tensor.matmul(p1[:, j, :], a8_slice(n, hc), ones8p,
                                 start=False, stop=True, perf_mode=DR)
            nc.scalar.activation(h1[:, 2 * hp:2 * hp + 2, :], p1[:],
                                 AF.Lrelu, alpha=NEG_SLOPE)

        p2 = l2ps.tile([128, KC, M], F32, tag="p2")
        for kc in range(KC):
            for hp in range(HC // 2):
                nc.tensor.matmul(
                    p2[:, kc, :],
                    w2_sb[:, 2 * hp:2 * hp + 2, kc * 128:(kc + 1) * 128],
                    h1[:, 2 * hp:2 * hp + 2, :],
                    start=(hp == 0), stop=False, perf_mode=DR)
            nc.tensor.matmul(p2[:, kc, :], b8_slice(kc), ones8p,
                             start=False, stop=True, perf_mode=DR)
        h2 = h2p.tile([128, KC, M], F8, tag="h2")
        nc.scalar.activation(h2[:], p2[:], AF.Lrelu, alpha=NEG_SLOPE)

        for mc in range(MC):
            nc.tensor.matmul(
                d_ps[:, mc, n:n + 1],
                h2[:, 0:2, mc * 128:(mc + 1) * 128],
                w3p[:, 0:2, :],
                start=True, stop=True, perf_mode=DR)

    # ---- epilogue ----
    e_t = ep.tile([128, MC, NL], F32, tag="e_t")
    nc.scalar.activation(e_t[:], d_ps[:], AF.Exp, bias=c0_sb, scale=-1.0)
    w_t = ep.tile([128, MC, NL], F32, tag="w_t")
    nc.vector.tensor_mul(w_t[:], e_t[:], cost_t[:])
    efg_t = ep.tile([128, MC, NL], F32, tag="efg_t")
    nc.gpsimd.tensor_mul(efg_t[:], e_t[:], mask_bc[:])
    wfg_t = ep.tile([128, MC, NL], F32, tag="wfg_t")
    nc.gpsimd.tensor_mul(wfg_t[:], w_t[:], mask_bc[:])

    stats_sb = consts.tile([128, 4, MC], F32, tag="stats_sb")
    for s, src in ((0, e_t), (1, efg_t), (2, w_t), (3, wfg_t)):
        nc.vector.tensor_reduce(stats_sb[:, s, :], src[:],
                                axis=mybir.AxisListType.X,
                                op=mybir.AluOpType.add)
    nc.sync.dma_start(stats[:], stats_sb[:])


def build_program():
    nc = bacc.Bacc("TRN2", target_bir_lowering=False, debug=False,
                   num_devices=N_CORES)
    pa = nc.dram_tensor("pa", [128, _PA_COLS], F32, kind="ExternalInput").ap()
    pq = nc.dram_tensor("pq", [128, _PQ_COLS], BF16, kind="ExternalInput").ap()
    p8 = nc.dram_tensor("p8", [128, _P8_COLS], F8, kind="ExternalInput").ap()
    pz = nc.dram_tensor("pz", [1, _PZ_COLS], F8, kind="ExternalInput").ap()
    stats = nc.dram_tensor("stats", [128, 4, MC], F32,
                           kind="ExternalOutput").ap()

    with tile.TileContext(nc) as tc, ExitStack() as ctx:
        consts = ctx.enter_context(tc.tile_pool(name="consts", bufs=1))
        pa_sb = consts.tile([128, _PA_COLS], F32, tag="pa_sb")
        nc.sync.dma_start(pa_sb[:], pa[:])
        pq_sb = consts.tile([128, _PQ_COLS], BF16, tag="pq_sb")
        nc.sync.dma_start(pq_sb[:, 0:_QH0], pq[:, 0:_QH0])
        nc.sync.dma_start(pq_sb[:, _QH0:_MC0], pq[:, _QH0:_MC0])
        p8_sb = consts.tile([128, _P8_COLS], F8, tag="p8_sb")
        nc.sync.dma_start(p8_sb[:], p8[:])
        pz_sb = consts.tile([128, _PZ_COLS], F8, tag="pz_sb")
        nc.sync.dma_start(pz_sb[0:1, :], pz[:])
        nc.sync.dma_start(pq_sb[:, _MC0:_PQ_COLS], pq[:, _MC0:_PQ_COLS])
        emit_body(nc, tc, ctx, pa_sb, pq_sb, p8_sb, pz_sb, stats)

    nc.compile()
    return nc


def shard_inputs(M_aug, Q, W1, b1, W2, b2, W3, b3, N_fg):
    """Host-side layout prep. Returns per-core input maps."""
    import ml_dtypes
    f = np.float32
    bf = ml_dtypes.bfloat16
    e4 = ml_dtypes.float8_e4m3
    M_aug = np.asarray(M_aug, np.float64)
    Q = np.asarray(Q, np.float64)
    W1 = np.asarray(W1, np.float64)
    W2 = np.asarray(W2, np.float64)
    W3 = np.asarray(W3, np.float64)
    b1 = np.asarray(b1, np.float64)
    b2 = np.asarray(b2, np.float64)
    b3 = np.asarray(b3, np.float64)
    nfg = int(N_fg)

    def part_major(a2d, chunks):  # [C*128, F] -> [128, C*F]
        cdim, fdim = a2d.shape
        assert cdim == chunks * 128
        return np.ascontiguousarray(
            a2d.reshape(chunks, 128, fdim).transpose(1, 0, 2)).reshape(128, -1)

    v_chunks = np.array(_V_CHUNKS)
    w1_scale = np.where(np.repeat(v_chunks, 128), 2.0, 1.0)  # [D]
    v_dmask = np.repeat(v_chunks, 128).astype(np.float64)     # [D]

    p8_v = np.zeros((128, _P8_COLS), e4)
    p8_v[:, _W10:_W20] = part_major((W1 * w1_scale[None, :]).T, DC).astype(e4)
    p8_v[:, _W20:_W30] = part_major(W2.T, HC).astype(e4)
    p8_v[:, _W30:_P8_COLS] = W3.reshape(KC, 128).T.astype(e4)
    p8_v = np.ascontiguousarray(p8_v)

    qt = part_major(Q.T, DC).astype(bf)                  # [128, DC*M] bf16
    qh = (0.5 * qt.astype(np.float64) ** 2).astype(bf)

    base = np.zeros((128, _PA_COLS), f)
    base[:, _C00] = -float(b3[0])
    base[0, _ON0:_ON0 + 128] = 1.0

    in_maps = []
    for c in range(N_CORES):
        rows = slice(c * NL, (c + 1) * NL)
        Mrows = M_aug[rows]                               # [NL, D]
        pa_v = base.copy()
        pa_v[:, _MT0:_C00] = part_major(-Mrows.T, DC)
        pa_v[0, _MN0:_MN0 + NL] = (Mrows ** 2).sum(-1)
        gidx = np.arange(c * NL, (c + 1) * NL)
        pa_v[0, _FG0:_FG0 + MC * NL] = np.tile((gidx < nfg).astype(f), MC)

        # A' bias: b1 + sum_{d in v-chunks} W1[h,d]*M[n,d]^2, [H, NL]
        Ap = (b1[:, None] + W1 @ (v_dmask[:, None] * (Mrows.T ** 2)))
        pz_v = np.zeros((1, _PZ_COLS), e4)
        # pairs [data(128) | zeros(128)]; A region == Ap.T [NL*HC, 128]
        pz_v[0, _A80:_B80].reshape(NL * HC, 2, 128)[:, 0, :] = \
            np.ascontiguousarray(Ap.T).reshape(NL * HC, 128).astype(e4)
        pz_v[0, _B80:_O80].reshape(KC, 2, 128)[:, 0, :] = \
            b2.reshape(KC, 128).astype(e4)
        pz_v[0, _O80:_PZ_COLS] = 1.0

        pq_v = np.zeros((128, _PQ_COLS), bf)
        pq_v[:, _QT0:_QH0] = qt
        pq_v[:, _QH0:_MC0] = qh
        pq_v[:, _MC0:_PQ_COLS] = part_major(-2.0 * Mrows.T, DC).astype(bf)
        in_maps.append({"pa": pa_v, "pq": np.ascontiguousarray(pq_v),
                        "p8": p8_v, "pz": pz_v})
    return in_maps


def combine(stats_list, Q):
    """stats_list: per-core [128, 4, MC] arrays -> (score, score_fg)."""
    st = np.stack([
        np.asarray(s, np.float64).transpose(1, 2, 0).reshape(4, M)
        for s in stats_list
    ])  # [C, 4, M]
    S1 = st[:, 0].sum(0)
    S1fg = st[:, 1].sum(0)
    Sc = st[:, 2].sum(0)
    Scfg = st[:, 3].sum(0)
    qn2 = (np.asarray(Q, np.float64) ** 2).sum(-1)
    score = Sc / S1 + qn2
    score_fg = Scfg / S1 + qn2 * (S1fg / S1)
    return score.astype(np.float32), score_fg.astype(np.float32)


_PROGRAM_CACHE = {}


def run(trace=False, **inputs):
    if "prog" not in _PROGRAM_CACHE:
        _PROGRAM_CACHE["prog"] = build_program()
    nc = _PROGRAM_CACHE["prog"]
    in_maps = shard_inputs(**inputs)
    res = run_bass_kernel_spmd(nc, in_maps, list(range(N_CORES)), trace=trace)
    outs = combine([res.results[c]["stats"] for c in range(N_CORES)],
                   inputs["Q"])
    return outs, res


def kernel(**inputs):
    outs, _ = run(trace=False, **inputs)
    return outs
